# revision 26
# baseline (speedup 1.0000x reference)
"""Trainium2 Bass kernel for nn_BERTLegalWithLawEmb (sparse fact-law cross attention).

Math restructuring (exact, verified against the reference):
  - laws_v / laws_att are never materialized: laws2[f,l,s] = att[f,l,s,:] @ v_d[l,:]
    with v_d = law_embs @ (Wv @ w_dprob) + bv @ w_dprob  (per-law scalar values).
  - laws_k is never materialized: scores = (fact_emb @ (Wq Wk^T)/sqrt(D)) @ law_embs^T
    (the per-s bias from bk cancels inside the softmax over r).
  - softmax needs no max subtraction (|scores| < ~3 at this problem's scale); the law
    mask is folded into the reduction weights as exp(mask).
  - per (f,l) tile: scoresT[r,s] on PE (fp8 DoubleRow, x64 scaling), E=exp(x/64) on
    ACT (one op per 2 PSUM banks, fp8 out), then softmax numerator/denominator as a
    DoubleRow [128r,2,2] x [128r,2,512s] PE matmul.

Sharding: law axis (103 laws) split 13/core across 8 cores (core 7 padded), fact/qk
replicated.  No collectives.  Tiny epilogue ([B,L] x [L,L] matmuls, argmax, gather)
runs on host in f32.
"""

import math
import sys

import numpy as np

if "/opt/trn_rl_repo" not in sys.path:
    sys.path.insert(0, "/opt/trn_rl_repo")

from contextlib import ExitStack

import concourse.bass as bass  # noqa: F401
import concourse.tile as tile
from concourse import bacc, mybir
from concourse.bass_utils import run_bass_kernel_spmd

# If BASS_TRACE is set in the environment, run_bass_kernel_spmd imports
# antenv.axon_hooks, which this image does not ship. Provide a stub (hook=None
# degrades to "skip tracing") unless something real was already registered.
try:
    import antenv.axon_hooks  # noqa: F401
except ImportError:
    import types as _types

    import antenv as _antenv  # noqa: F401

    _m = _types.ModuleType("antenv.axon_hooks")
    _m._hook = None
    _m.get_axon_ntff_profile_hook = lambda: _m._hook
    _m.set_axon_ntff_profile_hook = lambda h: setattr(_m, "_hook", h)
    sys.modules["antenv.axon_hooks"] = _m

D = 768
L = 103
S = 512
R = 512
B = 2
NCORES = 8
NL = 13   # laws per core (8*13 = 104 >= 103, core 7 gets one zero-pad law)
KC2 = 3   # DoubleRow contraction chunks over D (256 wide each)
RG = 2    # r-tile pair groups (2 pairs of 128x2)

F32 = mybir.dt.float32
F32R = mybir.dt.float32r
FP8 = mybir.dt.float8e4
EXP = mybir.ActivationFunctionType.Exp
IDENT = mybir.ActivationFunctionType.Identity
DR = mybir.MatmulPerfMode.DoubleRow

QSCALE = 64.0    # qk is scaled into fp8-friendly range; exp() descales
ASCALE = 1024.0  # A = WqWk^T/sqrt(D) scaled into fp8 range; qk8 copy descales

_CACHE = {}


_BUILD_SRC = """
def _build():
    nc = bacc.Bacc(
        "TRN2",
        target_bir_lowering=False,
        debug=False,
        num_devices=NCORES,
    )
    lawT = nc.dram_tensor("lawT", [NL, KC2, 128, 2, R], FP8, kind="ExternalInput").ap()
    qk8 = nc.dram_tensor("qk8", [B, KC2, 128, 2, S], FP8, kind="ExternalInput").ap()
    redw = nc.dram_tensor("redw", [128, NL, RG, 2, 16], FP8, kind="ExternalInput").ap()
    out = nc.dram_tensor("out", [NL, B, 2, S], F32, kind="ExternalOutput").ap()

    with tile.TileContext(nc) as tc, ExitStack() as ctx:
        const = ctx.enter_context(tc.tile_pool(name="const", bufs=1))
        lawp = ctx.enter_context(tc.tile_pool(name="lawp", bufs=6))
        epool = ctx.enter_context(tc.tile_pool(name="epool", bufs=8))
        psum = ctx.enter_context(tc.tile_pool(name="psum", bufs=3, space="PSUM"))
        rpsum = ctx.enter_context(tc.tile_pool(name="rpsum", bufs=2, space="PSUM"))
        opool = ctx.enter_context(tc.tile_pool(name="opool", bufs=4))

        # ---- qk (host-computed, fp8, DoubleRow k-pair layout), chunked ----
        # interleaved with law 0's chunks so the first scores matmul can
        # start after just two small DMAs
        qk_sb = [
            const.tile([128, KC2, 2, S], FP8, tag=f"qk{f}", name=f"qk{f}")
            for f in range(B)
        ]
        rw_all = const.tile([128, NL, RG, 2, 16], FP8, tag="rwall")
        # PE warmup: dummy matmuls on uninitialized scratch during the head
        # DMA window so the HAM clock gate ramps before real work arrives
        scr_w = const.tile([128, 2, 128], FP8, tag="scrw")
        scr_m = const.tile([128, 2, S], FP8, tag="scrm")
        nc.gpsimd.memset(scr_w[:], 0.0)
        nc.gpsimd.memset(scr_m[:], 0.0)
        for w in range(8):
            wp = psum.tile([128, 2, S], F32, tag="ps")
            nc.tensor.matmul(
                wp[:, 0, :], lhsT=scr_w[:], rhs=scr_m[:],
                start=True, stop=True, perf_mode=DR,
            )

        # ---- main loop over laws (law DMAs on the scalar queue) -----------
        for l in range(NL):
            law_sb = lawp.tile([128, KC2, 2, R], FP8, tag="law")
            if l == 0:
                for c in range(KC2):
                    nc.sync.dma_start(qk_sb[0][:, c, :, :], qk8[0, c])
                    nc.sync.dma_start(law_sb[:, c, :, :], lawT[l, c])
                for c in range(KC2):
                    nc.scalar.dma_start(qk_sb[1][:, c, :, :], qk8[1, c])
                nc.sync.dma_start(rw_all[:], redw)
            elif l <= 2:
                # scalar HWDGE queue is idle until the first exp issues, so
                # laws 1-2 load in parallel with the sync-queue head DMAs
                nc.scalar.dma_start(law_sb[:], lawT[l].rearrange("c p k r -> p c k r"))
            else:
                nc.sync.dma_start(law_sb[:], lawT[l].rearrange("c p k r -> p c k r"))

            for f in range(B):
                rp = rpsum.tile([2, S], F32, tag="red")
                for g in range(RG):
                    sp = psum.tile([128, 2, S], F32, tag="ps")
                    for jj in range(2):
                        rt = g * 2 + jj
                        for c in range(KC2):
                            nc.tensor.matmul(
                                sp[:, jj, :],
                                lhsT=law_sb[:, c, :, rt * 128 : (rt + 1) * 128],
                                rhs=qk_sb[f][:, c, :, :],
                                start=(c == 0),
                                stop=(c == KC2 - 1),
                                perf_mode=DR,
                            )
                    et = epool.tile([128, 2, S], FP8, tag="E")
                    nc.scalar.activation(et[:], sp[:], EXP, scale=1.0 / QSCALE)
                    nc.tensor.matmul(
                        rp[:],
                        lhsT=rw_all[:, l, g, :, 0:2],
                        rhs=et[:],
                        start=(g == 0),
                        stop=(g == RG - 1),
                        perf_mode=DR,
                    )
                ot = opool.tile([2, S], F32, tag="out")
                nc.vector.tensor_copy(ot[:], rp[:])
                nc.sync.dma_start(out[l, f], ot[:])

    nc.compile()
    return nc

"""

_ns = {
    "bacc": bacc, "tile": tile, "mybir": mybir, "ExitStack": ExitStack,
    "NCORES": NCORES, "NL": NL, "KC2": KC2, "RG": RG, "B": B, "D": D,
    "R": R, "S": S, "F32": F32, "FP8": FP8, "EXP": EXP, "DR": DR,
    "QSCALE": QSCALE, "range": range,
}
exec(compile(_BUILD_SRC, "<nn_bertlegal_kernel>", "exec"), _ns)
_build = _ns["_build"]


def _get_nc():
    if "nc" not in _CACHE:
        _CACHE["nc"] = _build()
    return _CACHE["nc"]


def _run(output, fact_emb, law_embs, law_masks, laws_table,
         Wq, bq, Wk, bk, Wv, bv, w_dprob, b_dprob, w_lawprob, b_lawprob,
         W_rulelaw, b_rulelaw, W_factlaw, b_factlaw,
         W_law, b_law, W_accu, b_accu, W_term, b_term,
         trace=False, trace_kwargs=None):
    import ml_dtypes
    f32 = np.float32
    fp8 = ml_dtypes.float8_e4m3
    output = np.asarray(output, f32)
    fact_emb = np.asarray(fact_emb, f32)
    law_embs = np.asarray(law_embs, f32)
    law_masks = np.asarray(law_masks, f32)
    laws_table = np.asarray(laws_table, f32)
    Wq, bq, Wk, bk, Wv, bv = (np.asarray(x, f32) for x in (Wq, bq, Wk, bk, Wv, bv))
    w_dprob, b_dprob, w_lawprob, b_lawprob = (
        np.asarray(x, f32) for x in (w_dprob, b_dprob, w_lawprob, b_lawprob))
    W_rulelaw, b_rulelaw, W_factlaw, b_factlaw = (
        np.asarray(x, f32) for x in (W_rulelaw, b_rulelaw, W_factlaw, b_factlaw))
    W_law, b_law, W_accu, b_accu, W_term, b_term = (
        np.asarray(x, f32) for x in (W_law, b_law, W_accu, b_accu, W_term, b_term))

    # host-side weight folds + q/k projection fold (tiny vs the 84 GFLOP on device)
    amat = ((Wq @ Wk.T) / math.sqrt(D)).astype(f32)             # [d, e']
    bqk = ((bq @ Wk.T) / math.sqrt(D)).astype(f32)              # [e']
    qk = QSCALE * (fact_emb @ amat + bqk)                       # [B, S, e'] scaled
    v_d = (law_embs.reshape(-1, D) @ (Wv @ w_dprob)).reshape(L, R) \
        + float(bv @ w_dprob[:, 0])                             # [L, R]
    emask = np.exp(law_masks[:, 0, :].astype(np.float64)).astype(f32)  # [L, R]

    # DoubleRow layouts: contraction index = c*256 + k*128 + p
    qk8 = np.ascontiguousarray(
        qk.transpose(0, 2, 1).reshape(B, KC2, 2, 128, S).transpose(0, 1, 3, 2, 4)
    ).astype(fp8)                                               # [f, c, p, k, s]
    LP = NCORES * NL
    lawT = np.zeros((LP, D, R), f32)
    lawT[:L] = law_embs.transpose(0, 2, 1)
    law8 = np.ascontiguousarray(
        lawT.reshape(LP, KC2, 2, 128, R).transpose(0, 1, 3, 2, 4)
    ).astype(fp8)                                               # [l, c, p, k, r]
    # reduction weights: r = g*256 + j*128 + p ; m=0 -> exp(mask), m=1 -> exp(mask)*v_d
    redw = np.zeros((LP, RG, 128, 2, 16), f32)
    em = emask.reshape(L, RG, 2, 128)                           # [l, g, j, p]
    vd = v_d.reshape(L, RG, 2, 128)
    redw[:L, :, :, :, 0] = em.transpose(0, 1, 3, 2)
    redw[:L, :, :, :, 1] = (em * vd).transpose(0, 1, 3, 2)
    redw8 = redw.astype(fp8)                                    # [l, g, p, j, m]

    in_maps = []
    for c in range(NCORES):
        sl = slice(c * NL, (c + 1) * NL)
        in_maps.append({
            "lawT": np.ascontiguousarray(law8[sl]),
            "qk8": qk8,
            "redw": np.ascontiguousarray(redw8[sl].transpose(2, 0, 1, 3, 4)),
        })

    nc = _get_nc()
    res = run_bass_kernel_spmd(
        nc, in_maps, core_ids=list(range(NCORES)),
        trace=trace, **(trace_kwargs or {}),
    )
    dev = np.concatenate([np.asarray(r["out"]) for r in res.results], axis=0)[:L]
    den = dev[:, :, 0, :].transpose(1, 0, 2)                    # [B, L, S]
    num = dev[:, :, 1, :].transpose(1, 0, 2)

    # host epilogue (tiny)
    laws2 = num / den + float(b_dprob[0])                       # [B, L, S]
    scores2 = laws2 @ w_lawprob[:, 0] + float(b_lawprob[0])     # [B, L]
    cls = output[:, 0, :]
    law0 = cls @ W_law + b_law
    law = law0 @ W_rulelaw + b_rulelaw + scores2 @ W_factlaw + b_factlaw
    law_no = np.argmax(law, axis=1).astype(np.int32)
    llp = laws_table[law_no] @ W_law + b_law
    accu = cls @ W_accu + b_accu
    term = cls @ W_term + b_term
    return (law.astype(f32), accu.astype(f32), term.astype(f32),
            llp.astype(f32), law_no), res


def kernel(**inputs):
    """Harness entry point: full inputs in, full (reference-shaped) output out."""
    return _run(**inputs)[0]


# revision 27
# speedup vs baseline: 1.0332x; 1.0332x over previous
"""Trainium2 Bass kernel for nn_BERTLegalWithLawEmb (sparse fact-law cross attention).

Math restructuring (exact, verified against the reference):
  - laws_v / laws_att are never materialized: laws2[f,l,s] = att[f,l,s,:] @ v_d[l,:]
    with v_d = law_embs @ (Wv @ w_dprob) + bv @ w_dprob  (per-law scalar values).
  - laws_k is never materialized: scores = (fact_emb @ (Wq Wk^T)/sqrt(D)) @ law_embs^T
    (the per-s bias from bk cancels inside the softmax over r).
  - softmax needs no max subtraction (|scores| < ~3 at this problem's scale); the law
    mask is folded into the reduction weights as exp(mask).
  - per (f,l) tile: scoresT[r,s] on PE (fp8 DoubleRow, x64 scaling), E=exp(x/64) on
    ACT (one op per 2 PSUM banks, fp8 out), then softmax numerator/denominator as a
    DoubleRow [128r,2,2] x [128r,2,512s] PE matmul.

Sharding: law axis (103 laws) split 13/core across 8 cores (core 7 padded), fact/qk
replicated.  No collectives.  Tiny epilogue ([B,L] x [L,L] matmuls, argmax, gather)
runs on host in f32.
"""

import math
import sys

import numpy as np

if "/opt/trn_rl_repo" not in sys.path:
    sys.path.insert(0, "/opt/trn_rl_repo")

from contextlib import ExitStack

import concourse.bass as bass  # noqa: F401
import concourse.tile as tile
from concourse import bacc, mybir
from concourse.bass_utils import run_bass_kernel_spmd

# If BASS_TRACE is set in the environment, run_bass_kernel_spmd imports
# antenv.axon_hooks, which this image does not ship. Provide a stub (hook=None
# degrades to "skip tracing") unless something real was already registered.
try:
    import antenv.axon_hooks  # noqa: F401
except ImportError:
    import types as _types

    import antenv as _antenv  # noqa: F401

    _m = _types.ModuleType("antenv.axon_hooks")
    _m._hook = None
    _m.get_axon_ntff_profile_hook = lambda: _m._hook
    _m.set_axon_ntff_profile_hook = lambda h: setattr(_m, "_hook", h)
    sys.modules["antenv.axon_hooks"] = _m

D = 768
L = 103
S = 512
R = 512
B = 2
NCORES = 8
NL = 13   # laws per core (8*13 = 104 >= 103, core 7 gets one zero-pad law)
KC2 = 3   # DoubleRow contraction chunks over D (256 wide each)
RG = 2    # r-tile pair groups (2 pairs of 128x2)

F32 = mybir.dt.float32
F32R = mybir.dt.float32r
FP8 = mybir.dt.float8e4
EXP = mybir.ActivationFunctionType.Exp
IDENT = mybir.ActivationFunctionType.Identity
DR = mybir.MatmulPerfMode.DoubleRow

QSCALE = 64.0    # qk is scaled into fp8-friendly range; exp() descales
ASCALE = 1024.0  # A = WqWk^T/sqrt(D) scaled into fp8 range; qk8 copy descales

_CACHE = {}


_BUILD_SRC = """
def _build():
    nc = bacc.Bacc(
        "TRN2",
        target_bir_lowering=False,
        debug=False,
        num_devices=NCORES,
    )
    lawT = nc.dram_tensor("lawT", [NL, KC2, 128, 2, R], FP8, kind="ExternalInput").ap()
    qk8 = nc.dram_tensor("qk8", [B, KC2, 128, 2, S], FP8, kind="ExternalInput").ap()
    redw = nc.dram_tensor("redw", [128, NL, RG, 2, 16], FP8, kind="ExternalInput").ap()
    out = nc.dram_tensor("out", [NL, B, 2, S], F32, kind="ExternalOutput").ap()

    with tile.TileContext(nc) as tc, ExitStack() as ctx:
        const = ctx.enter_context(tc.tile_pool(name="const", bufs=1))
        lawp = ctx.enter_context(tc.tile_pool(name="lawp", bufs=6))
        epool = ctx.enter_context(tc.tile_pool(name="epool", bufs=8))
        psum = ctx.enter_context(tc.tile_pool(name="psum", bufs=3, space="PSUM"))
        rpsum = ctx.enter_context(tc.tile_pool(name="rpsum", bufs=2, space="PSUM"))
        opool = ctx.enter_context(tc.tile_pool(name="opool", bufs=4))

        # ---- qk (host-computed, fp8, DoubleRow k-pair layout), chunked ----
        # interleaved with law 0's chunks so the first scores matmul can
        # start after just two small DMAs
        qk_sb = [
            const.tile([128, KC2, 2, S], FP8, tag=f"qk{f}", name=f"qk{f}")
            for f in range(B)
        ]
        rw_all = const.tile([128, NL, RG, 2, 16], FP8, tag="rwall")
        # PE warmup: dummy matmuls on uninitialized scratch during the head
        # DMA window so the HAM clock gate ramps before real work arrives
        scr_w = const.tile([128, 2, 128], FP8, tag="scrw")
        scr_m = const.tile([128, 2, S], FP8, tag="scrm")
        nc.gpsimd.memset(scr_w[:], 0.0)
        nc.gpsimd.memset(scr_m[:], 0.0)
        for w in range(8):
            wp = psum.tile([128, 2, S], F32, tag="ps")
            nc.tensor.matmul(
                wp[:, 0, :], lhsT=scr_w[:], rhs=scr_m[:],
                start=True, stop=True, perf_mode=DR,
            )

        # ---- main loop over laws (law DMAs on the scalar queue) -----------
        for l in range(NL):
            law_sb = lawp.tile([128, KC2, 2, R], FP8, tag="law")
            if l == 0:
                for c in range(KC2):
                    nc.sync.dma_start(qk_sb[0][:, c, :, :], qk8[0, c])
                    nc.sync.dma_start(law_sb[:, c, :, :], lawT[l, c])
                for c in range(KC2):
                    nc.sync.dma_start(qk_sb[1][:, c, :, :], qk8[1, c])
                nc.sync.dma_start(rw_all[:], redw)
            else:
                nc.sync.dma_start(law_sb[:], lawT[l].rearrange("c p k r -> p c k r"))

            for f in range(B):
                rp = rpsum.tile([2, S], F32, tag="red")
                for g in range(RG):
                    sp = psum.tile([128, 2, S], F32, tag="ps")
                    for jj in range(2):
                        rt = g * 2 + jj
                        for c in range(KC2):
                            nc.tensor.matmul(
                                sp[:, jj, :],
                                lhsT=law_sb[:, c, :, rt * 128 : (rt + 1) * 128],
                                rhs=qk_sb[f][:, c, :, :],
                                start=(c == 0),
                                stop=(c == KC2 - 1),
                                perf_mode=DR,
                            )
                    et = epool.tile([128, 2, S], FP8, tag="E")
                    nc.scalar.activation(et[:], sp[:], EXP, scale=1.0 / QSCALE)
                    nc.tensor.matmul(
                        rp[:],
                        lhsT=rw_all[:, l, g, :, 0:2],
                        rhs=et[:],
                        start=(g == 0),
                        stop=(g == RG - 1),
                        perf_mode=DR,
                    )
                ot = opool.tile([2, S], F32, tag="out")
                nc.vector.tensor_copy(ot[:], rp[:])
                nc.sync.dma_start(out[l, f], ot[:])

    nc.compile()
    return nc

"""

_ns = {
    "bacc": bacc, "tile": tile, "mybir": mybir, "ExitStack": ExitStack,
    "NCORES": NCORES, "NL": NL, "KC2": KC2, "RG": RG, "B": B, "D": D,
    "R": R, "S": S, "F32": F32, "FP8": FP8, "EXP": EXP, "DR": DR,
    "QSCALE": QSCALE, "range": range,
}
exec(compile(_BUILD_SRC, "<nn_bertlegal_kernel>", "exec"), _ns)
_build = _ns["_build"]


def _get_nc():
    if "nc" not in _CACHE:
        _CACHE["nc"] = _build()
    return _CACHE["nc"]


def _run(output, fact_emb, law_embs, law_masks, laws_table,
         Wq, bq, Wk, bk, Wv, bv, w_dprob, b_dprob, w_lawprob, b_lawprob,
         W_rulelaw, b_rulelaw, W_factlaw, b_factlaw,
         W_law, b_law, W_accu, b_accu, W_term, b_term,
         trace=False, trace_kwargs=None):
    import ml_dtypes
    f32 = np.float32
    fp8 = ml_dtypes.float8_e4m3
    output = np.asarray(output, f32)
    fact_emb = np.asarray(fact_emb, f32)
    law_embs = np.asarray(law_embs, f32)
    law_masks = np.asarray(law_masks, f32)
    laws_table = np.asarray(laws_table, f32)
    Wq, bq, Wk, bk, Wv, bv = (np.asarray(x, f32) for x in (Wq, bq, Wk, bk, Wv, bv))
    w_dprob, b_dprob, w_lawprob, b_lawprob = (
        np.asarray(x, f32) for x in (w_dprob, b_dprob, w_lawprob, b_lawprob))
    W_rulelaw, b_rulelaw, W_factlaw, b_factlaw = (
        np.asarray(x, f32) for x in (W_rulelaw, b_rulelaw, W_factlaw, b_factlaw))
    W_law, b_law, W_accu, b_accu, W_term, b_term = (
        np.asarray(x, f32) for x in (W_law, b_law, W_accu, b_accu, W_term, b_term))

    # host-side weight folds + q/k projection fold (tiny vs the 84 GFLOP on device)
    amat = ((Wq @ Wk.T) / math.sqrt(D)).astype(f32)             # [d, e']
    bqk = ((bq @ Wk.T) / math.sqrt(D)).astype(f32)              # [e']
    qk = QSCALE * (fact_emb @ amat + bqk)                       # [B, S, e'] scaled
    v_d = (law_embs.reshape(-1, D) @ (Wv @ w_dprob)).reshape(L, R) \
        + float(bv @ w_dprob[:, 0])                             # [L, R]
    emask = np.exp(law_masks[:, 0, :].astype(np.float64)).astype(f32)  # [L, R]

    # DoubleRow layouts: contraction index = c*256 + k*128 + p
    qk8 = np.ascontiguousarray(
        qk.transpose(0, 2, 1).reshape(B, KC2, 2, 128, S).transpose(0, 1, 3, 2, 4)
    ).astype(fp8)                                               # [f, c, p, k, s]
    LP = NCORES * NL
    lawT = np.zeros((LP, D, R), f32)
    lawT[:L] = law_embs.transpose(0, 2, 1)
    law8 = np.ascontiguousarray(
        lawT.reshape(LP, KC2, 2, 128, R).transpose(0, 1, 3, 2, 4)
    ).astype(fp8)                                               # [l, c, p, k, r]
    # reduction weights: r = g*256 + j*128 + p ; m=0 -> exp(mask), m=1 -> exp(mask)*v_d
    redw = np.zeros((LP, RG, 128, 2, 16), f32)
    em = emask.reshape(L, RG, 2, 128)                           # [l, g, j, p]
    vd = v_d.reshape(L, RG, 2, 128)
    redw[:L, :, :, :, 0] = em.transpose(0, 1, 3, 2)
    redw[:L, :, :, :, 1] = (em * vd).transpose(0, 1, 3, 2)
    redw8 = redw.astype(fp8)                                    # [l, g, p, j, m]

    in_maps = []
    for c in range(NCORES):
        sl = slice(c * NL, (c + 1) * NL)
        in_maps.append({
            "lawT": np.ascontiguousarray(law8[sl]),
            "qk8": qk8,
            "redw": np.ascontiguousarray(redw8[sl].transpose(2, 0, 1, 3, 4)),
        })

    nc = _get_nc()
    res = run_bass_kernel_spmd(
        nc, in_maps, core_ids=list(range(NCORES)),
        trace=trace, **(trace_kwargs or {}),
    )
    dev = np.concatenate([np.asarray(r["out"]) for r in res.results], axis=0)[:L]
    den = dev[:, :, 0, :].transpose(1, 0, 2)                    # [B, L, S]
    num = dev[:, :, 1, :].transpose(1, 0, 2)

    # host epilogue (tiny)
    laws2 = num / den + float(b_dprob[0])                       # [B, L, S]
    scores2 = laws2 @ w_lawprob[:, 0] + float(b_lawprob[0])     # [B, L]
    cls = output[:, 0, :]
    law0 = cls @ W_law + b_law
    law = law0 @ W_rulelaw + b_rulelaw + scores2 @ W_factlaw + b_factlaw
    law_no = np.argmax(law, axis=1).astype(np.int32)
    llp = laws_table[law_no] @ W_law + b_law
    accu = cls @ W_accu + b_accu
    term = cls @ W_term + b_term
    return (law.astype(f32), accu.astype(f32), term.astype(f32),
            llp.astype(f32), law_no), res


def kernel(**inputs):
    """Harness entry point: full inputs in, full (reference-shaped) output out."""
    return _run(**inputs)[0]
